# revision 1
# baseline (speedup 1.0000x reference)
"""TransformerConv GNN message passing on 8 TRN2 NeuronCores (Bass/Tile).

Strategy (graph/edge parallelism, dst-sharded — no collectives needed):
  - Core c owns destination nodes [c*6250, (c+1)*6250); edges are sharded by
    their dst node, so the segment-softmax and scatter-aggregation are fully
    core-local (the per-node max/sum all-reduce from the hint is avoided by
    making every dst's edges land on one core).
  - Per the sharding hint, edges ship with their GATHERED node features:
    the host packs x[src], x[dst], edge_attr and the dst-onehot per edge
    (bf16, pre-transposed per 128-edge sub-chunk) into one fused stream.
  - On device, per dst-window of 128 nodes, per group of <=4 sub-chunks:
      kve = xsrcT.T@[Wk|Wv] + eaT.T@[We|We]  (PE, PSUM accumulates k+e | v+e)
      qd  = xdstT.T@Wq                        (PE)
      alpha = rowsum_per_head(qd * kve.k)     (DVE)
      pe  = exp(alpha/8)                      (ACT, softmax max-shift dropped:
                                               mathematically identical)
      ve  = kve.v * pe ; [ve | pe] scatter:   agg[128,130] += onehot.T @ ve
    Window epilogue: out = (agg/denom) @ Wproj + x_own @ (Wskip@Wproj) + bias.
  - Softmax normalization is applied after aggregation (linearity), padding
    edges carry an all-zero onehot row so they contribute nothing.

kernel(**inputs) takes the FULL unsharded inputs and returns the FULL
[50000, 128] float32 output.  Set TRACE=True to capture NTFF timing
(LAST_EXEC_TIME_NS / LAST_RESULTS are populated).
"""
import sys
from contextlib import ExitStack

import numpy as np

for _p in ('/opt/trn_rl_repo', '/root/.axon_site/_ro/trn_rl_repo'):
    if _p not in sys.path:
        sys.path.append(_p)

import ml_dtypes

import concourse.bass as bass          # noqa: E402
import concourse.mybir as mybir        # noqa: E402
import concourse.tile as tile          # noqa: E402
from concourse import bacc             # noqa: E402
from concourse import bass_utils       # noqa: E402

bf16 = ml_dtypes.bfloat16
F32 = mybir.dt.float32
BF16 = mybir.dt.bfloat16

N = 50000
E = 800000
DIM = 128
H = 2
C = 64
P = 128
NCORES = 8
NODES_PER_CORE = N // NCORES          # 6250
WIN = 128
NWIN = (NODES_PER_CORE + WIN - 1) // WIN   # 49
NODES_PAD = NWIN * WIN                # 6272
GROUP = 4
ALPHA_SCALE = 0.125                   # 1/sqrt(64)

TRACE = False
LAST_EXEC_TIME_NS = None
LAST_RESULTS = None


# ----------------------------------------------------------------------------
# host-side sharding / preprocessing
# ----------------------------------------------------------------------------

def _schedule(S):
    groups = []
    off = 0
    sub_base = 0
    for w in range(NWIN):
        for g0 in range(0, S[w], GROUP):
            Wg = min(GROUP, S[w] - g0)
            groups.append((w, sub_base + g0, Wg, off))
            off += Wg * 512
        sub_base += S[w]
    return groups, off


def _prep(x, edge_attr, edge_index):
    x_np = np.asarray(x, dtype=np.float32)
    src = np.asarray(edge_index[0], dtype=np.int64)
    dst = np.asarray(edge_index[1], dtype=np.int64)

    core_of = dst // NODES_PER_CORE
    dst_local = dst - core_of * NODES_PER_CORE
    win_of = dst_local // WIN

    counts = np.zeros((NCORES, NWIN), dtype=np.int64)
    np.add.at(counts, (core_of, win_of), 1)
    S = np.maximum(np.ceil(counts / 128).astype(np.int64).max(axis=0), 1)
    TS = int(S.sum())
    EPAD = TS * 128

    order = np.lexsort((np.arange(E), win_of, core_of))
    run_ends = np.cumsum(counts.reshape(-1))
    run_starts = np.concatenate([[0], run_ends[:-1]]).reshape(NCORES, NWIN)
    run_ends = run_ends.reshape(NCORES, NWIN)
    wbase = np.concatenate([[0], np.cumsum(S)])

    groups, total_cols = _schedule(S.tolist())

    ea_np = np.asarray(edge_attr, dtype=np.float32)
    per_core = []
    for c in range(NCORES):
        src_pad = np.zeros(EPAD, dtype=np.int64)
        dstg_pad = np.zeros(EPAD, dtype=np.int64)
        dstoh_pad = np.full(EPAD, -1, dtype=np.int64)
        ea_rows = np.zeros(EPAD, dtype=np.int64)
        ea_valid = np.zeros(EPAD, dtype=bool)
        for w in range(NWIN):
            sel = order[run_starts[c, w]:run_ends[c, w]]
            cnt = len(sel)
            base = int(wbase[w]) * 128
            src_pad[base:base + cnt] = src[sel]
            dstg_pad[base:base + cnt] = dst[sel]
            dstoh_pad[base:base + cnt] = dst_local[sel] - w * WIN
            ea_rows[base:base + cnt] = sel
            ea_valid[base:base + cnt] = True

        ea = np.zeros((EPAD, DIM), dtype=np.float32)
        ea[ea_valid] = ea_np[ea_rows[ea_valid]]
        xs = x_np[src_pad]
        xd = x_np[dstg_pad]
        oh = np.zeros((EPAD, 128), dtype=np.float32)
        vmask = dstoh_pad >= 0
        oh[np.nonzero(vmask)[0], dstoh_pad[vmask]] = 1.0

        def sub_t(mat):   # feature dim on partitions, per 128-edge sub-chunk
            return mat.reshape(TS, 128, 128).transpose(2, 0, 1).reshape(128, EPAD)

        def sub_n(mat):   # edges on partitions (onehot)
            return mat.reshape(TS, 128, 128).transpose(1, 0, 2).reshape(128, EPAD)

        comp = [sub_t(ea), sub_t(xs), sub_t(xd), sub_n(oh)]
        edge_pm = np.empty((128, total_cols), dtype=bf16)
        for (_w, s0, Wg, off) in groups:
            for k in range(4):
                edge_pm[:, off + k * Wg * 128: off + (k + 1) * Wg * 128] = \
                    comp[k][:, s0 * 128:(s0 + Wg) * 128].astype(bf16)
        per_core.append(edge_pm)

    return per_core, dict(S=S.tolist(), TS=TS)


def _device_inputs(inputs):
    x = np.asarray(inputs['x'], dtype=np.float32)
    per_core, sched = _prep(x, inputs['edge_attr'], inputs['edge_index'])
    ident = np.eye(128, dtype=np.float32).astype(bf16)
    biases = {k: np.asarray(inputs[k], dtype=np.float32)
              for k in ['bq', 'bk', 'bv', 'bskip', 'bproj']}
    has_bias = any(np.any(b != 0) for b in biases.values())
    in_maps = []
    for c in range(NCORES):
        own = np.zeros((NODES_PAD, DIM), dtype=np.float32)
        own[:NODES_PER_CORE] = x[c * NODES_PER_CORE:(c + 1) * NODES_PER_CORE]
        m = dict(
            edge_pm=per_core[c],
            xTown_pm=np.ascontiguousarray(own.T).astype(bf16),
            ident_in=ident,
            wq=np.asarray(inputs['Wq'], dtype=np.float32),
            wk=np.asarray(inputs['Wk'], dtype=np.float32),
            wv=np.asarray(inputs['Wv'], dtype=np.float32),
            we=np.asarray(inputs['We'], dtype=np.float32),
            wskip=np.asarray(inputs['Wskip'], dtype=np.float32),
            wproj=np.asarray(inputs['Wproj'], dtype=np.float32),
        )
        if has_bias:
            m['bkv_row'] = np.ascontiguousarray(
                np.concatenate([biases['bk'], biases['bv']])[None, :])
            m['bq_row'] = np.ascontiguousarray(biases['bq'][None, :])
            m['bskip_col'] = np.ascontiguousarray(biases['bskip'][:, None])
            m['bproj_row'] = np.ascontiguousarray(biases['bproj'][None, :])
        in_maps.append(m)
    return sched, in_maps, has_bias


# ----------------------------------------------------------------------------
# device kernel
# ----------------------------------------------------------------------------

def _build(sched, has_bias=False):
    S = sched['S']
    groups, total_cols = _schedule(S)
    nc = bacc.Bacc("TRN2", target_bir_lowering=False, debug=False)

    edge_pm = nc.dram_tensor("edge_pm", [P, total_cols], BF16, kind="ExternalInput").ap()
    xTown_pm = nc.dram_tensor("xTown_pm", [P, NODES_PAD], BF16, kind="ExternalInput").ap()
    ident_in = nc.dram_tensor("ident_in", [P, P], BF16, kind="ExternalInput").ap()
    w_in = {}
    for name in ["wq", "wk", "wv", "we", "wskip", "wproj"]:
        w_in[name] = nc.dram_tensor(name, [P, P], F32, kind="ExternalInput").ap()
    if has_bias:
        bkv_row = nc.dram_tensor("bkv_row", [1, 2 * P], F32, kind="ExternalInput").ap()
        bq_row = nc.dram_tensor("bq_row", [1, P], F32, kind="ExternalInput").ap()
        bskip_col = nc.dram_tensor("bskip_col", [P, 1], F32, kind="ExternalInput").ap()
        bproj_row = nc.dram_tensor("bproj_row", [1, P], F32, kind="ExternalInput").ap()
    out = nc.dram_tensor("out", [NODES_PAD, DIM], F32, kind="ExternalOutput").ap()

    with tile.TileContext(nc) as tc, ExitStack() as top:
        res = top.enter_context(tc.tile_pool(name="res", bufs=1))

        xTown_sb = res.tile([P, NODES_PAD], BF16)
        nc.sync.dma_start(out=xTown_sb[:], in_=xTown_pm[:, :])
        ident = res.tile([P, P], BF16)
        nc.sync.dma_start(out=ident[:], in_=ident_in[:, :])

        wsb = {}
        for name in ["wq", "wk", "wv", "we", "wskip", "wproj"]:
            wf = res.tile([P, P], F32, tag="wf32")
            nc.sync.dma_start(out=wf[:], in_=w_in[name][:, :])
            wb = res.tile([P, P], BF16, tag=f"{name}_b")
            nc.vector.tensor_copy(out=wb[:], in_=wf[:])
            wsb[name] = wb
        wkv_sb = res.tile([P, 2 * P], BF16)   # [Wk | Wv]
        nc.vector.tensor_copy(out=wkv_sb[:, 0:P], in_=wsb["wk"][:])
        nc.vector.tensor_copy(out=wkv_sb[:, P:2 * P], in_=wsb["wv"][:])
        wee_sb = res.tile([P, 2 * P], BF16)   # [We | We]
        nc.vector.tensor_copy(out=wee_sb[:, 0:P], in_=wsb["we"][:])
        nc.vector.tensor_copy(out=wee_sb[:, P:2 * P], in_=wsb["we"][:])

        if has_bias:
            bkv_sb = res.tile([1, 2 * P], BF16)
            bq_sb = res.tile([1, P], BF16)
            ones_row = res.tile([1, P], BF16)
            nc.vector.memset(ones_row[:], 1.0)
            bkvf = res.tile([1, 2 * P], F32)
            nc.sync.dma_start(out=bkvf[:], in_=bkv_row[:, :])
            nc.vector.tensor_copy(out=bkv_sb[:], in_=bkvf[:])
            bqf = res.tile([1, P], F32)
            nc.sync.dma_start(out=bqf[:], in_=bq_row[:, :])
            nc.vector.tensor_copy(out=bq_sb[:], in_=bqf[:])
            bskipc = res.tile([P, 1], F32)
            nc.sync.dma_start(out=bskipc[:], in_=bskip_col[:, :])
            bskipc_b = res.tile([P, 1], BF16)
            nc.vector.tensor_copy(out=bskipc_b[:], in_=bskipc[:])
            bprojf = res.tile([1, P], F32)
            nc.sync.dma_start(out=bprojf[:], in_=bproj_row[:, :])

        # fused skip weight: Wfused = Wskip @ Wproj  (and fused bias)
        wfused_sb = res.tile([P, P], BF16)
        bfused_sb = res.tile([1, P], BF16, name="bfused_sb") if has_bias else None
        with tc.tile_pool(name="wset_ps", bufs=1, space="PSUM") as wps_pool, \
             tc.tile_pool(name="wset_sb", bufs=1) as wsb_pool:
            tp = wps_pool.tile([P, P], BF16)
            nc.tensor.transpose(out=tp[:], in_=wsb["wskip"][:], identity=ident[:])
            wskipT = wsb_pool.tile([P, P], BF16)
            nc.vector.tensor_copy(out=wskipT[:], in_=tp[:])
            wf_ps = wps_pool.tile([P, P], F32)
            nc.tensor.matmul(out=wf_ps[:], lhsT=wskipT[:], rhs=wsb["wproj"][:],
                             start=True, stop=True)
            nc.vector.tensor_copy(out=wfused_sb[:], in_=wf_ps[:])
            if has_bias:
                bf_ps = wps_pool.tile([1, P], F32)
                nc.tensor.matmul(out=bf_ps[:], lhsT=bskipc_b[:], rhs=wsb["wproj"][:],
                                 start=True, stop=True)
                bff = wsb_pool.tile([1, P], F32)
                nc.vector.tensor_add(out=bff[:], in0=bf_ps[:], in1=bprojf[:])
                nc.vector.tensor_copy(out=bfused_sb[:], in_=bff[:])

        # ---------------- main loop (3-stage software pipeline) -------------
        with tc.tile_pool(name="edge_in", bufs=12) as in_pool, \
             tc.tile_pool(name="work", bufs=10) as wk_pool, \
             tc.tile_pool(name="kve_ps", bufs=3, space="PSUM") as kve_pool, \
             tc.tile_pool(name="qd_ps", bufs=1, space="PSUM") as qd_pool, \
             tc.tile_pool(name="agg_ps", bufs=1, space="PSUM") as agg_pool, \
             tc.tile_pool(name="outp", bufs=8) as out_pool:
            aggs = {}

            def epilogue(w):
                agg = aggs.pop(w)
                den = out_pool.tile([P, H], F32, tag="den", name=f"den{w}")
                nc.vector.tensor_scalar_add(den[:], agg[:, 128:130], 1e-30)
                inv = out_pool.tile([P, H], F32, tag="inv", name=f"inv{w}")
                nc.vector.reciprocal(out=inv[:], in_=den[:])
                aggn = out_pool.tile([P, P], BF16, tag="aggn", name=f"aggn{w}")
                nc.vector.tensor_mul(
                    out=aggn[:].rearrange("p (h c) -> p h c", c=C),
                    in0=agg[:, 0:P].rearrange("p (h c) -> p h c", c=C),
                    in1=inv[:].unsqueeze(2).broadcast_to([P, H, C]))
                tp_ps = agg_pool.tile([P, P], BF16, tag="agg", name=f"tp{w}")
                nc.tensor.transpose(out=tp_ps[:], in_=aggn[:], identity=ident[:])
                aggT = out_pool.tile([P, P], BF16, tag="aggT", name=f"aggT{w}")
                nc.scalar.copy(out=aggT[:], in_=tp_ps[:])
                fin = agg_pool.tile([P, P], F32, tag="agg", name=f"fin{w}")
                nc.tensor.matmul(out=fin[:], lhsT=aggT[:], rhs=wsb["wproj"][:],
                                 start=True, stop=False, skip_group_check=True)
                nc.tensor.matmul(out=fin[:], lhsT=xTown_sb[:, w * P:(w + 1) * P],
                                 rhs=wfused_sb[:], start=False,
                                 stop=not has_bias, skip_group_check=True)
                if has_bias:
                    nc.tensor.matmul(out=fin[:], lhsT=ones_row[:], rhs=bfused_sb[:],
                                     start=False, stop=True, skip_group_check=True)
                fin_sb = out_pool.tile([P, P], F32, tag="fin_sb", name=f"fsb{w}")
                nc.scalar.copy(out=fin_sb[:], in_=fin[:])
                nc.sync.dma_start(out=out[w * P:(w + 1) * P, :], in_=fin_sb[:])

            def scatter(pend):
                w, s0, Wg, ve, oh_in = pend
                Sw = S[w]
                wstart = sum(S[:w])
                for j in range(Wg):
                    nd = s0 - wstart + j
                    nc.tensor.matmul(
                        out=aggs[w][:], lhsT=oh_in[:, j * P:(j + 1) * P],
                        rhs=ve[:, j, :],
                        start=(nd == 0), stop=(nd == Sw - 1),
                        skip_group_check=True)
                if s0 - wstart + Wg == Sw:
                    epilogue(w)

            def stage_C(st):
                Wg = st['Wg']
                qk = wk_pool.tile([P, Wg, P], BF16, tag="qk", name=f"qk{st['s0']}")
                nc.vector.tensor_mul(out=qk[:], in0=st['qd_sb'][:],
                                     in1=st['kve'][:, 0:Wg, 0:P])
                alpha = wk_pool.tile([P, Wg, H], F32, tag="alpha",
                                     name=f"al{st['s0']}")
                nc.vector.reduce_sum(
                    out=alpha[:],
                    in_=qk[:].rearrange("p j (h c) -> p (j h) c", c=C),
                    axis=mybir.AxisListType.X)
                pe_x = wk_pool.tile([P, Wg, P], BF16, tag="pe_x",
                                    name=f"pe{st['s0']}")
                nc.scalar.activation(
                    out=pe_x[:],
                    in_=alpha[:].unsqueeze(3).broadcast_to([P, Wg, H, C]),
                    func=mybir.ActivationFunctionType.Exp, scale=ALPHA_SCALE)
                st['alpha'] = alpha
                st['pe_x'] = pe_x

            def stage_D(st):
                Wg = st['Wg']
                ve = wk_pool.tile([P, Wg, 130], BF16, tag="ve", name=f"ve{st['s0']}")
                nc.vector.tensor_mul(out=ve[:, :, 0:P],
                                     in0=st['kve'][:, 0:Wg, P:2 * P],
                                     in1=st['pe_x'][:])
                nc.scalar.activation(
                    out=ve[:, :, P:P + H], in_=st['alpha'][:],
                    func=mybir.ActivationFunctionType.Exp, scale=ALPHA_SCALE)
                scatter((st['w'], st['s0'], Wg, ve, st['oh_in']))

            stC = None
            stD = None
            cur_w = -1
            for (w, s0, Wg, off) in groups:
                if w != cur_w:
                    cur_w = w
                    aggs[w] = agg_pool.tile([P, 130], F32, tag="agg", name=f"agg{w}")

                blk = in_pool.tile([P, Wg * 512], BF16, tag="blk")
                nc.sync.dma_start(out=blk[:], in_=edge_pm[:, off:off + Wg * 512])
                W128 = Wg * P
                ea_in = blk[:, 0:W128]
                xs_in = blk[:, W128:2 * W128]
                xd_in = blk[:, 2 * W128:3 * W128]
                oh_in = blk[:, 3 * W128:4 * W128]

                if stC is not None:
                    stage_C(stC)
                if stD is not None:
                    stage_D(stD)

                kve = kve_pool.tile([P, GROUP, 2 * P], F32, tag="kve")
                qd = qd_pool.tile([P, GROUP, P], F32, tag="qd")
                for j in range(Wg):
                    nc.tensor.matmul(out=qd[:, j, :],
                                     lhsT=xd_in[:, j * P:(j + 1) * P],
                                     rhs=wsb["wq"][:], start=True,
                                     stop=not has_bias, skip_group_check=True)
                    if has_bias:
                        nc.tensor.matmul(out=qd[:, j, :], lhsT=ones_row[:],
                                         rhs=bq_sb[:], start=False, stop=True,
                                         skip_group_check=True)
                for j in range(Wg):
                    nc.tensor.matmul(out=kve[:, j, :],
                                     lhsT=xs_in[:, j * P:(j + 1) * P],
                                     rhs=wkv_sb[:], start=True, stop=False,
                                     skip_group_check=True)
                    nc.tensor.matmul(out=kve[:, j, :],
                                     lhsT=ea_in[:, j * P:(j + 1) * P],
                                     rhs=wee_sb[:], start=False,
                                     stop=not has_bias, skip_group_check=True)
                    if has_bias:
                        nc.tensor.matmul(out=kve[:, j, :], lhsT=ones_row[:],
                                         rhs=bkv_sb[:], start=False, stop=True,
                                         skip_group_check=True)

                qd_sb = wk_pool.tile([P, Wg, P], BF16, tag="qd_sb")
                nc.scalar.copy(out=qd_sb[:], in_=qd[:, 0:Wg, :])

                stD = stC
                stC = dict(w=w, s0=s0, Wg=Wg, kve=kve, qd_sb=qd_sb, oh_in=oh_in)

            stage_C(stC)
            stage_D(stD)
            stage_D(stC)

    nc.compile()
    return nc


# ----------------------------------------------------------------------------
# entry point
# ----------------------------------------------------------------------------

def kernel(**inputs):
    global LAST_EXEC_TIME_NS, LAST_RESULTS
    assert np.asarray(inputs['x']).shape == (N, DIM)
    assert np.asarray(inputs['edge_index']).shape == (2, E)

    sched, in_maps, has_bias = _device_inputs(inputs)
    nc = _build(sched, has_bias=has_bias)
    res = bass_utils.run_bass_kernel_spmd(
        nc, in_maps, core_ids=list(range(NCORES)), trace=TRACE)
    LAST_EXEC_TIME_NS = res.exec_time_ns
    LAST_RESULTS = res
    outs = [r['out'][:NODES_PER_CORE] for r in res.results]
    return np.ascontiguousarray(
        np.concatenate(outs, axis=0).astype(np.float32))

